# revision 17
# baseline (speedup 1.0000x reference)
"""Trainium2 Bass kernel for nn_PlotLine (gaussian line rasterization).

reference semantics (per sample n of 8):
    5611 line points (31 segments x 181 lerp steps) each with (x, y, w);
    plot[l, axis, i] = exp(-(coord_l - i)^2 / (2 w_l)), i in 0..127
    out[n] = tanh(plotx^T @ ploty)           # [128, 128]

Strategy: pure data parallel, one sample per NeuronCore (8 cores).
Per core, all heavy math runs on-device:
  - For each 128-point chunk, U2(i) = (i - x)^2/(2w) is a quadratic in i with
    per-point coefficients.  We evaluate it with ONE bf16 tensor-engine matmul
    per group of 4 chunks (K=48 block-diagonal basis, N=512), using a 3-way
    bf16 split of the fp64 coefficients so the reconstruction is fp32-accurate
    (bf16 x bf16 products are exact; PSUM accumulates in fp32).
  - ScalarE does a single exp(-U2) activation pass per group (PSUM->SBUF,
    fp16 out) covering both axes (1024 cols).
  - 4 fp16 matmuls per group accumulate draft[i,j] += plotx^T @ ploty in PSUM.
  - Final tanh on ScalarE, DMA out.
Host (numpy) only prepares the tiny coefficient tensors (~320KB/core).
"""

from contextlib import ExitStack

import numpy as np
import ml_dtypes

import concourse.bacc as bacc
import concourse.bass as bass
import concourse.mybir as mybir
import concourse.tile as tile
from concourse.bass_utils import run_bass_kernel_spmd

bf16 = ml_dtypes.bfloat16

IMG = 128
CMP = 181            # interpolation steps per segment: int(128*sqrt(2))
NPTS = 32
NSEG = NPTS - 1
L = NSEG * CMP       # 5611 line points per sample
CH = 44              # chunks of 128 line points (padded)
LP = CH * 128        # 5632
GR = 11              # groups of 4 chunks
CEN = 64.0           # quadratic centering offset (reduces fp32 cancellation)
NCORES = 8


# ----------------------------------------------------------------- host prep

def _interp_lines(points):
    """points [N, 32, 3] f32 -> line [N, 5611, 3] f32 (matches reference)."""
    t = np.arange(CMP + 1, dtype=np.float32) / np.float32(CMP)
    p0 = points[:, :-1, None, :].astype(np.float32)
    p1 = points[:, 1:, None, :].astype(np.float32)
    tt = t[None, None, :CMP, None]
    line = (np.float32(1.0) - tt) * p0 + tt * p1
    return line.reshape(points.shape[0], L, 3)


def _split3(v):
    """3-way bf16 split of fp64 array: v ~= v1 + v2 + v3 with ~2^-25 rel err."""
    v1 = v.astype(bf16)
    r = v - v1.astype(np.float64)
    v2 = r.astype(bf16)
    r = r - v2.astype(np.float64)
    v3 = r.astype(bf16)
    return v1, v2, v3


def _build_basis():
    """Shared block-diagonal basis [48, 512] bf16.

    Block j (cols 128j..128j+127) rows 12j+k: [1,1,1, i',i',i', H,H,H, LO,LO,LO]
    where i' = i - 64, H = bf16(i'^2), LO = i'^2 - H (exact in bf16).
    """
    basis = np.zeros((48, 512), dtype=bf16)
    i = np.arange(128, dtype=np.float64)
    ip = i - CEN
    H = (ip * ip).astype(bf16)
    LOW = (ip * ip - H.astype(np.float64)).astype(bf16)
    for j in range(4):
        cols = slice(128 * j, 128 * (j + 1))
        for k in range(3):
            basis[12 * j + 0 + k, cols] = bf16(1.0)
            basis[12 * j + 3 + k, cols] = ip.astype(bf16)
            basis[12 * j + 6 + k, cols] = H
            basis[12 * j + 9 + k, cols] = LOW
    return basis


def _build_coefs(coord, w):
    """coord, w: [L] f32 -> coef [48, GR*128] bf16 for one axis of one sample.

    U2(i) = A + B i' + C i'^2,  A = x'^2/(2w), B = -2x'/(2w), C = 1/(2w),
    x' = x - 64, i' = i - 64.  Rows 12j+k of group-g columns hold the 3-way
    bf16 splits [A1..3, B1..3, C1..3, C1..3] for chunk 4g+j.
    """
    c64 = coord.astype(np.float64) - CEN
    r2 = 1.0 / (2.0 * w.astype(np.float64))
    A = c64 * c64 * r2
    B = -2.0 * c64 * r2
    C = r2
    pad = LP - L
    A = np.concatenate([A, np.full(pad, 1e6)]).reshape(CH, 128)
    B = np.concatenate([B, np.zeros(pad)]).reshape(CH, 128)
    C = np.concatenate([C, np.zeros(pad)]).reshape(CH, 128)
    A1, A2, A3 = _split3(A)
    B1, B2, B3 = _split3(B)
    C1, C2, C3 = _split3(C)
    rows = [A1, A2, A3, B1, B2, B3, C1, C2, C3, C1, C2, C3]
    coef = np.zeros((48, GR * 128), dtype=bf16)
    for g in range(GR):
        cols = slice(128 * g, 128 * (g + 1))
        for j in range(4):
            ch = 4 * g + j
            for k, arr in enumerate(rows):
                coef[12 * j + k, cols] = arr[ch]
    return coef


def _host_prep(points):
    """-> in_maps list of 8 dicts (one packed coef tensor per core).

    Column layout: [basis(512) | g0x(128) g0y(128) | g1x g1y | ... | g10y]
    so a prefix DMA delivers the basis plus the first groups.
    """
    line = _interp_lines(np.asarray(points, dtype=np.float32))
    basis = _build_basis()
    in_maps = []
    for n in range(points.shape[0]):
        cx = _build_coefs(line[n, :, 0], line[n, :, 2])
        cy = _build_coefs(line[n, :, 1], line[n, :, 2])
        parts = [basis]
        for g in range(GR):
            parts.append(cx[:, 128 * g:128 * (g + 1)])
            parts.append(cy[:, 128 * g:128 * (g + 1)])
        packed = np.concatenate(parts, axis=1)
        # pad to 128 rows of zeros: the device runs U2 matmuls with K=128
        # (full-row streams keep the PE HAM clock gate at 2.4GHz) and the
        # zero rows must be real zeros, not SBUF garbage
        full = np.zeros((128, packed.shape[1]), dtype=bf16)
        full[:48] = packed
        in_maps.append({"coefs": full})
    return in_maps


# ---------------------------------------------------------------- bass build

NCOLS = 512 + GR * 256   # basis | per-group (x|y) coef blocks
# DMA split points (columns): prefix delivers basis + first groups first
_DMA_SPLITS = [(0, 1024), (1024, 2048), (2048, NCOLS)]


def build_nc():
    nc = bacc.Bacc()
    coefs_d = nc.dram_tensor("coefs", [128, NCOLS], mybir.dt.bfloat16,
                             kind="ExternalInput")
    out_d = nc.dram_tensor("out", [128, 128], mybir.dt.float32,
                           kind="ExternalOutput")

    FP = mybir.dt
    with tile.TileContext(nc) as tc, ExitStack() as ctx:
        singles = ctx.enter_context(tc.tile_pool(name="singles", bufs=1))
        plots = ctx.enter_context(tc.tile_pool(name="plots", bufs=3))
        psums = ctx.enter_context(tc.tile_pool(name="psums", bufs=3, space="PSUM"))
        draftp = ctx.enter_context(tc.tile_pool(name="draftp", bufs=1, space="PSUM"))

        # dummy exp pulls the ACT exp/tanh table load forward, under the DMA.
        # memsets on the otherwise-idle VectorE so nothing serializes behind
        # the ~1.3us ACT table load.
        dummy = singles.tile([128, 8], mybir.dt.float32)
        nc.vector.memset(dummy, 0)
        nc.scalar.activation(out=dummy, in_=dummy,
                             func=mybir.ActivationFunctionType.Exp, scale=-1.0)

        # bf16 warmup matmuls bridge the PE from the entry barrier until the
        # coef DMA lands (~3.5us): continuous full-row activity releases the
        # PE HAM clock gate (1.2 -> 2.4GHz) right as real work starts
        warm = singles.tile([128, 640], FP.bfloat16)
        nc.vector.memset(warm, 0)
        scratch = draftp.tile([128, 512], FP.float32, tag="scratch")
        for _ in range(8):
            nc.tensor.matmul(scratch, warm[:, 0:128], warm[:, 128:640],
                             start=True, stop=True)

        # pipelined coef load: 3 tiles so early groups start ASAP.
        # Full 128 rows (48 data + 80 host-provided zeros): K=128 U2 matmuls
        # stream all PE rows — partial-K matmuls keep the PE's HAM activity
        # monitor below threshold and the clock stays gated at 1.2GHz.
        parts = []
        for lo, hi in _DMA_SPLITS:
            p = singles.tile([128, hi - lo], FP.bfloat16, tag=f"coef{lo}")
            nc.sync.dma_start(out=p, in_=coefs_d[:, lo:hi])
            parts.append(p)

        def coef_slice(col):
            for (lo, hi), p in zip(_DMA_SPLITS, parts):
                if lo <= col and col + 128 <= hi:
                    return p[:, col - lo:col - lo + 128]
            raise AssertionError(col)

        basis = parts[0][:, 0:512]

        def cx(g):
            return coef_slice(512 + 256 * g)

        def cy(g):
            return coef_slice(512 + 256 * g + 128)

        draft = draftp.tile([128, 128], FP.float32, tag="draft")

        u2_tiles = []

        def emit_u2(g):
            u2 = psums.tile([128, 1024], FP.float32, tag="u2")
            nc.tensor.matmul(u2[:, 0:512], cx(g), basis, start=True, stop=True)
            nc.tensor.matmul(u2[:, 512:1024], cy(g), basis, start=True, stop=True)
            u2_tiles.append(u2)

        emit_u2(0)
        emit_u2(1)
        for g in range(GR):
            plot = plots.tile([128, 1024], FP.float16, tag="plot")
            nc.scalar.activation(out=plot, in_=u2_tiles[g],
                                 func=mybir.ActivationFunctionType.Exp,
                                 scale=-1.0)
            # keep PE busy on a later group's U2 while ScalarE runs exp
            if g + 2 < GR:
                emit_u2(g + 2)
            for j in range(4):
                nc.tensor.matmul(
                    draft,
                    plot[:, 128 * j:128 * (j + 1)],
                    plot[:, 512 + 128 * j:512 + 128 * (j + 1)],
                    start=(g == 0 and j == 0),
                    stop=(g == GR - 1 and j == 3),
                )

        outt = singles.tile([128, 128], FP.float32)
        nc.scalar.activation(out=outt, in_=draft,
                             func=mybir.ActivationFunctionType.Tanh)
        nc.sync.dma_start(out=out_d[:], in_=outt)
    nc.compile()
    return nc


# ----------------------------------------------------------------- interface

def _run(points, trace=False):
    points = np.asarray(points, dtype=np.float32)
    assert points.shape == (NCORES, NPTS, 3), points.shape
    in_maps = _host_prep(points)
    nc = build_nc()
    res = run_bass_kernel_spmd(nc, in_maps, core_ids=list(range(NCORES)),
                               trace=trace)
    out = np.stack([r["out"] for r in res.results], axis=0).astype(np.float32)
    return out, res


def kernel(points):
    out, _ = _run(points, trace=False)
    return out


if __name__ == "__main__":
    pts = np.random.default_rng(0).uniform(0, 1, (8, 32, 3)).astype(np.float32)
    pts[:, :, :2] *= IMG
    pts[:, :, 2] = pts[:, :, 2] * 2.5 + 0.5
    out = kernel(pts)
    print(out.shape, out.dtype, np.abs(out).max())


# revision 18
# speedup vs baseline: 1.0092x; 1.0092x over previous
"""Trainium2 Bass kernel for nn_PlotLine (gaussian line rasterization).

reference semantics (per sample n of 8):
    5611 line points (31 segments x 181 lerp steps) each with (x, y, w);
    plot[l, axis, i] = exp(-(coord_l - i)^2 / (2 w_l)), i in 0..127
    out[n] = tanh(plotx^T @ ploty)           # [128, 128]

Strategy: pure data parallel, one sample per NeuronCore (8 cores).
Per core, all heavy math runs on-device:
  - For each 128-point chunk, U2(i) = (i - x)^2/(2w) is a quadratic in i with
    per-point coefficients.  We evaluate it with ONE bf16 tensor-engine matmul
    per group of 4 chunks (K=48 block-diagonal basis, N=512), using a 3-way
    bf16 split of the fp64 coefficients so the reconstruction is fp32-accurate
    (bf16 x bf16 products are exact; PSUM accumulates in fp32).
  - ScalarE does a single exp(-U2) activation pass per group (PSUM->SBUF,
    fp16 out) covering both axes (1024 cols).
  - 4 fp16 matmuls per group accumulate draft[i,j] += plotx^T @ ploty in PSUM.
  - Final tanh on ScalarE, DMA out.
Host (numpy) only prepares the tiny coefficient tensors (~320KB/core).
"""

from contextlib import ExitStack

import numpy as np
import ml_dtypes

import concourse.bacc as bacc
import concourse.bass as bass
import concourse.mybir as mybir
import concourse.tile as tile
from concourse.bass_utils import run_bass_kernel_spmd

bf16 = ml_dtypes.bfloat16

IMG = 128
CMP = 181            # interpolation steps per segment: int(128*sqrt(2))
NPTS = 32
NSEG = NPTS - 1
L = NSEG * CMP       # 5611 line points per sample
CH = 44              # chunks of 128 line points (padded)
LP = CH * 128        # 5632
GR = 11              # groups of 4 chunks
CEN = 64.0           # quadratic centering offset (reduces fp32 cancellation)
NCORES = 8


# ----------------------------------------------------------------- host prep

def _interp_lines(points):
    """points [N, 32, 3] f32 -> line [N, 5611, 3] f32 (matches reference)."""
    t = np.arange(CMP + 1, dtype=np.float32) / np.float32(CMP)
    p0 = points[:, :-1, None, :].astype(np.float32)
    p1 = points[:, 1:, None, :].astype(np.float32)
    tt = t[None, None, :CMP, None]
    line = (np.float32(1.0) - tt) * p0 + tt * p1
    return line.reshape(points.shape[0], L, 3)


def _split3(v):
    """3-way bf16 split of fp64 array: v ~= v1 + v2 + v3 with ~2^-25 rel err."""
    v1 = v.astype(bf16)
    r = v - v1.astype(np.float64)
    v2 = r.astype(bf16)
    r = r - v2.astype(np.float64)
    v3 = r.astype(bf16)
    return v1, v2, v3


def _build_basis():
    """Shared block-diagonal basis [48, 512] bf16.

    Block j (cols 128j..128j+127) rows 12j+k: [1,1,1, i',i',i', H,H,H, LO,LO,LO]
    where i' = i - 64, H = bf16(i'^2), LO = i'^2 - H (exact in bf16).
    """
    basis = np.zeros((48, 512), dtype=bf16)
    i = np.arange(128, dtype=np.float64)
    ip = i - CEN
    H = (ip * ip).astype(bf16)
    LOW = (ip * ip - H.astype(np.float64)).astype(bf16)
    for j in range(4):
        cols = slice(128 * j, 128 * (j + 1))
        for k in range(3):
            basis[12 * j + 0 + k, cols] = bf16(1.0)
            basis[12 * j + 3 + k, cols] = ip.astype(bf16)
            basis[12 * j + 6 + k, cols] = H
            basis[12 * j + 9 + k, cols] = LOW
    return basis


def _build_coefs(coord, w):
    """coord, w: [L] f32 -> coef [48, GR*128] bf16 for one axis of one sample.

    U2(i) = A + B i' + C i'^2,  A = x'^2/(2w), B = -2x'/(2w), C = 1/(2w),
    x' = x - 64, i' = i - 64.  Rows 12j+k of group-g columns hold the 3-way
    bf16 splits [A1..3, B1..3, C1..3, C1..3] for chunk 4g+j.
    """
    c64 = coord.astype(np.float64) - CEN
    r2 = 1.0 / (2.0 * w.astype(np.float64))
    A = c64 * c64 * r2
    B = -2.0 * c64 * r2
    C = r2
    pad = LP - L
    A = np.concatenate([A, np.full(pad, 1e6)]).reshape(CH, 128)
    B = np.concatenate([B, np.zeros(pad)]).reshape(CH, 128)
    C = np.concatenate([C, np.zeros(pad)]).reshape(CH, 128)
    A1, A2, A3 = _split3(A)
    B1, B2, B3 = _split3(B)
    C1, C2, C3 = _split3(C)
    rows = [A1, A2, A3, B1, B2, B3, C1, C2, C3, C1, C2, C3]
    coef = np.zeros((48, GR * 128), dtype=bf16)
    for g in range(GR):
        cols = slice(128 * g, 128 * (g + 1))
        for j in range(4):
            ch = 4 * g + j
            for k, arr in enumerate(rows):
                coef[12 * j + k, cols] = arr[ch]
    return coef


def _host_prep(points):
    """-> in_maps list of 8 dicts (one packed coef tensor per core).

    Column layout: [basis(512) | g0x(128) g0y(128) | g1x g1y | ... | g10y]
    so a prefix DMA delivers the basis plus the first groups.
    """
    line = _interp_lines(np.asarray(points, dtype=np.float32))
    basis = _build_basis()
    in_maps = []
    for n in range(points.shape[0]):
        cx = _build_coefs(line[n, :, 0], line[n, :, 2])
        cy = _build_coefs(line[n, :, 1], line[n, :, 2])
        parts = [basis]
        for g in range(GR):
            parts.append(cx[:, 128 * g:128 * (g + 1)])
            parts.append(cy[:, 128 * g:128 * (g + 1)])
        packed = np.concatenate(parts, axis=1)
        # pad to 128 rows of zeros: the device runs U2 matmuls with K=128
        # (full-row streams keep the PE HAM clock gate at 2.4GHz) and the
        # zero rows must be real zeros, not SBUF garbage
        full = np.zeros((128, packed.shape[1]), dtype=bf16)
        full[:48] = packed
        in_maps.append({"coefs": full})
    return in_maps


# ---------------------------------------------------------------- bass build

NCOLS = 512 + GR * 256   # basis | per-group (x|y) coef blocks
# DMA split points (columns): prefix delivers basis + first groups first
_DMA_SPLITS = [(0, 1024), (1024, 2048), (2048, NCOLS)]


def build_nc():
    nc = bacc.Bacc()
    coefs_d = nc.dram_tensor("coefs", [128, NCOLS], mybir.dt.bfloat16,
                             kind="ExternalInput")
    out_d = nc.dram_tensor("out", [128, 128], mybir.dt.float32,
                           kind="ExternalOutput")

    FP = mybir.dt
    with tile.TileContext(nc) as tc, ExitStack() as ctx:
        singles = ctx.enter_context(tc.tile_pool(name="singles", bufs=1))
        plots = ctx.enter_context(tc.tile_pool(name="plots", bufs=3))
        psums = ctx.enter_context(tc.tile_pool(name="psums", bufs=3, space="PSUM"))
        draftp = ctx.enter_context(tc.tile_pool(name="draftp", bufs=1, space="PSUM"))

        # dummy exp pulls the ACT exp/tanh table load forward, under the DMA.
        # memsets on the otherwise-idle VectorE so nothing serializes behind
        # the ~1.3us ACT table load.
        dummy = singles.tile([128, 8], mybir.dt.float32)
        nc.vector.memset(dummy, 0)
        nc.scalar.activation(out=dummy, in_=dummy,
                             func=mybir.ActivationFunctionType.Exp, scale=-1.0)



        # pipelined coef load: 3 tiles so early groups start ASAP.
        # Full 128 rows (48 data + 80 host-provided zeros): K=128 U2 matmuls
        # stream all PE rows — partial-K matmuls keep the PE's HAM activity
        # monitor below threshold and the clock stays gated at 1.2GHz.
        parts = []
        for lo, hi in _DMA_SPLITS:
            p = singles.tile([128, hi - lo], FP.bfloat16, tag=f"coef{lo}")
            nc.sync.dma_start(out=p, in_=coefs_d[:, lo:hi])
            parts.append(p)

        def coef_slice(col):
            for (lo, hi), p in zip(_DMA_SPLITS, parts):
                if lo <= col and col + 128 <= hi:
                    return p[:, col - lo:col - lo + 128]
            raise AssertionError(col)

        basis = parts[0][:, 0:512]

        def cx(g):
            return coef_slice(512 + 256 * g)

        def cy(g):
            return coef_slice(512 + 256 * g + 128)

        draft = draftp.tile([128, 128], FP.float32, tag="draft")

        u2_tiles = []

        def emit_u2(g):
            u2 = psums.tile([128, 1024], FP.float32, tag="u2")
            nc.tensor.matmul(u2[:, 0:512], cx(g), basis, start=True, stop=True)
            nc.tensor.matmul(u2[:, 512:1024], cy(g), basis, start=True, stop=True)
            u2_tiles.append(u2)

        emit_u2(0)
        emit_u2(1)
        for g in range(GR):
            plot = plots.tile([128, 1024], FP.float16, tag="plot")
            nc.scalar.activation(out=plot, in_=u2_tiles[g],
                                 func=mybir.ActivationFunctionType.Exp,
                                 scale=-1.0)
            # keep PE busy on a later group's U2 while ScalarE runs exp
            if g + 2 < GR:
                emit_u2(g + 2)
            for j in range(4):
                nc.tensor.matmul(
                    draft,
                    plot[:, 128 * j:128 * (j + 1)],
                    plot[:, 512 + 128 * j:512 + 128 * (j + 1)],
                    start=(g == 0 and j == 0),
                    stop=(g == GR - 1 and j == 3),
                )

        outt = singles.tile([128, 128], FP.float32)
        nc.scalar.activation(out=outt, in_=draft,
                             func=mybir.ActivationFunctionType.Tanh)
        nc.sync.dma_start(out=out_d[:], in_=outt)
    nc.compile()
    return nc


# ----------------------------------------------------------------- interface

def _run(points, trace=False):
    points = np.asarray(points, dtype=np.float32)
    assert points.shape == (NCORES, NPTS, 3), points.shape
    in_maps = _host_prep(points)
    nc = build_nc()
    res = run_bass_kernel_spmd(nc, in_maps, core_ids=list(range(NCORES)),
                               trace=trace)
    out = np.stack([r["out"] for r in res.results], axis=0).astype(np.float32)
    return out, res


def kernel(points):
    out, _ = _run(points, trace=False)
    return out


if __name__ == "__main__":
    pts = np.random.default_rng(0).uniform(0, 1, (8, 32, 3)).astype(np.float32)
    pts[:, :, :2] *= IMG
    pts[:, :, 2] = pts[:, :, 2] * 2.5 + 0.5
    out = kernel(pts)
    print(out.shape, out.dtype, np.abs(out).max())
